# revision 1
# baseline (speedup 1.0000x reference)
"""Trainium2 Bass kernel for nn_CategoricalLinear (MoE-routing batched matvec).

Problem: out[b] = weight[selected_ids[b]] @ x[b]
  x: [2048, 512] f32, selected_ids: [2048] int, weight: [64, 512, 512] f32
  out: [2048, 512] f32

Strategy (category-sharded, NOT the data-parallel hint):
  - Host: stable-sort samples by category; category c's samples become a
    contiguous block. Transpose x so features lie on SBUF partitions.
  - Each of the 8 cores owns 8 categories (8 MB weight slab — the minimal
    1/8 slice of the 64 MB table) and ALL samples routed to them (~256).
  - Per category g: out_g[s, o] = sum_i x[s, i] * W_g[o, i] computed as
    4 accumulating PE matmuls: stationary = xT chunk [128(K=IN), PC(samples)],
    moving = W_g^T chunk [128(K=IN), 512(OUT)], PSUM [PC, 512].
    float32r data path -> full-rate PE (fp32 would stream at 1/4 rate).
  - Weight slab streamed per-category (1 MB DMAs) and double-buffered so the
    PE and the output path hide entirely under the weight DMA (~8 MB/core,
    the bandwidth floor for this sharding).
  - Host: unpad + inverse-permute rows back to the original sample order.

This is better than data-parallel replication: sharding the batch would make
every core read ~the whole 64 MB table (8x the aggregate HBM traffic) and
leaves ~4 samples per (core, category) matmul.
"""

import numpy as np

B, IN, OUT, C = 2048, 512, 512, 64
NCORES = 8
CPC = C // NCORES  # categories per core
KCH = IN // 128  # contraction chunks of 128


def _build_nc(
    PC,
    mm_dtype: str = "float32r",
    loop_iters: int = 0,
    unroll: int = 1,
    wbufs: int = 4,
    cats_per_dma: int = 1,
    interleave: bool = False,
    alt_rings: bool = False,
    split_first: bool = False,
    w_engine: str = "sync",
    merge_xt: bool = False,
    ppbufs: int = 4,
    opbufs: int = 3,
):
    """Build + compile the SPMD Bass program (same NEFF runs on all 8 cores).

    PC: per-slot sample capacities (even, <= 128) — an int (uniform) or a
        sequence of CPC values. Slot g on every core holds one category
        padded to PC[g] samples.
    loop_iters: if > 0, wrap the body in a device-side For_i loop with
        `unroll` copies of the body per iteration (timing use only).
    """
    import concourse.mybir as mybir
    import concourse.tile as tile
    from concourse import bacc

    f32 = mybir.dt.float32
    mmdt = getattr(mybir.dt, mm_dtype)
    PCs = [PC] * CPC if isinstance(PC, int) else list(PC)
    assert len(PCs) == CPC
    SOFF = [0]
    for p in PCs:
        SOFF.append(SOFF[-1] + p)
    NCOL = SOFF[-1]

    nc = bacc.Bacc(
        "TRN2", target_bir_lowering=False, debug=False, num_devices=NCORES
    )
    wt = nc.dram_tensor("wt", [CPC * IN, OUT], mmdt, kind="ExternalInput").ap()
    xt = nc.dram_tensor("xt", [IN, NCOL], mmdt, kind="ExternalInput").ap()
    out = nc.dram_tensor("out", [NCOL, OUT], f32, kind="ExternalOutput").ap()

    with tile.TileContext(nc) as tc:
        with (
            tc.tile_pool(name="xp", bufs=1) as xp,
            tc.tile_pool(name="wp", bufs=wbufs) as wp,
            tc.tile_pool(name="pp", bufs=ppbufs, space="PSUM") as pp,
            tc.tile_pool(name="op", bufs=opbufs) as op,
        ):

            def body():
                G = cats_per_dma
                if interleave:
                    # p-outer row mapping: partition p holds IN rows
                    # KCH*p + s (s=0..KCH-1). Every DMA is contiguous per
                    # partition (8 KB weight runs, one single xT DMA); the
                    # contraction over s-subsets is a row permutation the
                    # matmul accumulation doesn't care about, as long as x
                    # and W use the same mapping.
                    xt4 = xp.tile([128, KCH, NCOL], mmdt, tag="x4")
                    nc.scalar.dma_start(
                        out=xt4[:], in_=xt.rearrange("(p s) c -> p s c", p=128)
                    )
                    lhs = lambda s, g: xt4[:, s, SOFF[g] : SOFF[g] + PCs[g]]
                elif merge_xt:
                    # One 3-D DMA for all four k-chunks (same k-outer layout,
                    # one descriptor chain / one fixed cost on the fill path).
                    xt1 = xp.tile([128, KCH, NCOL], mmdt, tag="x1")
                    nc.scalar.dma_start(
                        out=xt1[:], in_=xt.rearrange("(k p) c -> p k c", p=128)
                    )
                    lhs = lambda s, g: xt1[:, s, SOFF[g] : SOFF[g] + PCs[g]]
                else:
                    xts = []
                    for k in range(KCH):
                        t = xp.tile([128, NCOL], mmdt, tag=f"x{k}")
                        # ACT ring: keep SP HWDGE free for the weight stream
                        nc.scalar.dma_start(
                            out=t[:], in_=xt[k * 128 : (k + 1) * 128, :]
                        )
                        xts.append(t)
                    lhs = lambda s, g: xts[s][:, SOFF[g] : SOFF[g] + PCs[g]]
                for gp in range(0, CPC, G):
                    # Weight block [G cats] as SBUF [128, G, KCH, OUT]. G MB/DMA.
                    wtile = wp.tile([128, G, KCH, OUT], mmdt)
                    if interleave:
                        src = wt[gp * IN : (gp + G) * IN, :].rearrange(
                            "(g p s) o -> p g s o", p=128, s=KCH
                        )
                    else:
                        src = wt[gp * IN : (gp + G) * IN, :].rearrange(
                            "(g k p) o -> p g k o", p=128, k=KCH
                        )
                    weng = (
                        nc.scalar
                        if (alt_rings and (gp // G) % 2)
                        else getattr(nc, w_engine)
                    )
                    if split_first and gp == 0 and G == 1:
                        # Halve the fill latency: the first two matmuls only
                        # need k-chunks 0-1, so land them in their own DMA.
                        half = wp.tile([128, 1, KCH // 2, OUT], mmdt, tag="wh")
                        weng.dma_start(
                            out=half[:],
                            in_=wt[0 : IN // 2, :].rearrange(
                                "(g k p) o -> p g k o", p=128, k=KCH // 2
                            ),
                        )
                        weng.dma_start(
                            out=wtile[:, :, KCH // 2 :, :],
                            in_=wt[IN // 2 : IN, :].rearrange(
                                "(g k p) o -> p g k o", p=128, k=KCH // 2
                            ),
                        )
                        first_half = half
                    else:
                        weng.dma_start(out=wtile[:], in_=src)
                        first_half = None
                    for gl in range(G):
                        g = gp + gl
                        ps = pp.tile([PCs[g], OUT], f32, tag="ps")
                        for k in range(KCH):
                            if first_half is not None and k < KCH // 2:
                                rhs = first_half[:, gl, k, :]
                            else:
                                rhs = wtile[:, gl, k, :]
                            nc.tensor.matmul(
                                ps[:],
                                lhsT=lhs(k, g),
                                rhs=rhs,
                                start=(k == 0),
                                stop=(k == KCH - 1),
                            )
                        ot = op.tile([PCs[g], OUT], f32, tag="ot")
                        nc.vector.tensor_copy(out=ot[:], in_=ps[:])
                        nc.scalar.dma_start(
                            out=out[SOFF[g] : SOFF[g] + PCs[g], :], in_=ot[:]
                        )

            if loop_iters > 0:
                with tc.For_i(0, loop_iters, 1):
                    for _ in range(unroll):
                        body()
            else:
                for _ in range(unroll):
                    body()
    nc.compile()
    return nc


def _prepare(x, selected_ids, weight, mm_dtype="float32r"):
    """Host-side shard prep. Returns (in_maps, meta), or (None, None) when the
    inputs don't fit the compiled layout (handled by the host fallback)."""
    host_dt = np.float16 if mm_dtype == "float16" else np.float32
    x = np.ascontiguousarray(np.asarray(x, dtype=np.float32))
    ids = np.asarray(selected_ids).astype(np.int64).ravel()
    weight = np.asarray(weight, dtype=np.float32)
    if ids.size != B or ids.min() < 0 or ids.max() >= C:
        return None, None  # out-of-range ids -> host path
    counts = np.bincount(ids, minlength=C)
    mx = int(counts.max())
    if mx > 128 or weight.shape != (C, OUT, IN) or x.shape != (B, IN):
        return None, None  # pathological skew / unexpected shape -> host path
    order = np.argsort(ids, kind="stable")
    x_sorted = x[order]
    offs = np.zeros(C + 1, np.int64)
    offs[1:] = np.cumsum(counts)
    # Identity assignment, uniform capacity rounded to 16. Measured fastest on
    # HW: sorted-assignment layouts with tighter per-slot capacities moved
    # ~0.5 MB/core less but ran 0.9-1.5 us slower (shorter DMA runs / smaller
    # output blocks cost more than the saved bytes). Capacity must be EVEN or
    # the fp32r matmul fast path degrades ~2x (PC=43 measured 58.8 us).
    assign = np.arange(C).reshape(NCORES, CPC).T  # [slot, core] -> category
    PCs = [min(128, max(16, (mx + 15) // 16 * 16))] * CPC
    SOFF = np.zeros(CPC + 1, np.int64)
    SOFF[1:] = np.cumsum(PCs)
    NCOL = int(SOFF[-1])
    wt_t = np.ascontiguousarray(weight.transpose(0, 2, 1).astype(host_dt))
    in_maps = []
    for core in range(NCORES):
        xt_k = np.zeros((IN, NCOL), host_dt)
        wlist = []
        for g in range(CPC):
            c = int(assign[g, core])
            n = int(counts[c])
            if n:
                xt_k[:, SOFF[g] : SOFF[g] + n] = (
                    x_sorted[offs[c] : offs[c + 1]].T.astype(host_dt)
                )
            wlist.append(wt_t[c])
        w_k = np.concatenate(wlist, axis=0)  # [CPC*IN, OUT]
        in_maps.append({"wt": w_k, "xt": xt_k})
    meta = dict(
        PCs=PCs, SOFF=SOFF, assign=assign, counts=counts, offs=offs, order=order
    )
    return in_maps, meta


def _gather(results, meta):
    counts, offs, order = meta["counts"], meta["offs"], meta["order"]
    assign, SOFF = meta["assign"], meta["SOFF"]
    out_sorted = np.empty((B, OUT), np.float32)
    for core in range(NCORES):
        o = results[core]["out"]
        for g in range(CPC):
            c = int(assign[g, core])
            n = int(counts[c])
            if n:
                out_sorted[offs[c] : offs[c + 1]] = o[SOFF[g] : SOFF[g] + n]
    out_full = np.empty_like(out_sorted)
    out_full[order] = out_sorted
    return out_full


_LAST = {}  # debug/test introspection: last built nc + shard maps


def kernel(x, selected_ids, weight):
    in_maps, meta = _prepare(x, selected_ids, weight)
    if in_maps is None:
        # Host fallback for inputs outside the compiled layout's assumptions.
        ids = np.asarray(selected_ids).astype(np.int64).ravel()
        w = np.asarray(weight, dtype=np.float32)
        xx = np.asarray(x, dtype=np.float32).reshape(ids.size, -1)
        outf = np.empty((ids.size, w.shape[1]), np.float32)
        for c in np.unique(ids):
            m = ids == c
            outf[m] = xx[m] @ w[c].T
        return outf
    from concourse.bass_utils import run_bass_kernel_spmd

    # float32: exact f32-class result (fro 1.2e-07 vs f64), measured 34.66 us
    # with wbufs=6 (vs float32r's 32.22 us / 1.25e-04 and float16's
    # 21.89 us / 2.50e-04). Exactness buys zero numerical-threshold risk.
    # wbufs=6: fp32 is PE-paced (4 cyc/row), so deeper weight lookahead wins
    # (-0.7 us); in the DMA-paced fp32r/fp16 regimes it measured worse.
    nc = _build_nc(meta["PCs"], mm_dtype="float32", wbufs=6)
    _LAST.update(nc=nc, in_maps=in_maps, meta=meta)
    res = run_bass_kernel_spmd(nc, in_maps, core_ids=list(range(NCORES)))
    return _gather(res.results, meta)



# revision 33
# speedup vs baseline: 2.2603x; 2.2603x over previous
"""Trainium2 Bass kernel for nn_CategoricalLinear (MoE-routing batched matvec).

Problem: out[b] = weight[selected_ids[b]] @ x[b]
  x: [2048, 512] f32, selected_ids: [2048] int, weight: [64, 512, 512] f32
  out: [2048, 512] f32

Strategy (category-sharded, NOT the data-parallel hint):
  - Host: stable-sort samples by category; category c's samples become a
    contiguous block. Transpose x so features lie on SBUF partitions.
  - Each of the 8 cores owns 8 categories (8 MB weight slab — the minimal
    1/8 slice of the 64 MB table) and ALL samples routed to them (~256).
  - Per category g: out_g[s, o] = sum_i x[s, i] * W_g[o, i] computed as
    4 accumulating PE matmuls: stationary = xT chunk [128(K=IN), PC(samples)],
    moving = W_g^T chunk [128(K=IN), 512(OUT)], PSUM [PC, 512].
    float32r data path -> full-rate PE (fp32 would stream at 1/4 rate).
  - Weight slab streamed per-category (1 MB DMAs) and double-buffered so the
    PE and the output path hide entirely under the weight DMA (~8 MB/core,
    the bandwidth floor for this sharding).
  - Host: unpad + inverse-permute rows back to the original sample order.

This is better than data-parallel replication: sharding the batch would make
every core read ~the whole 64 MB table (8x the aggregate HBM traffic) and
leaves ~4 samples per (core, category) matmul.
"""

import numpy as np

B, IN, OUT, C = 2048, 512, 512, 64
NCORES = 8
CPC = C // NCORES  # categories per core
KCH = IN // 128  # contraction chunks of 128


def _build_nc(
    PC,
    mm_dtype: str = "float32r",
    loop_iters: int = 0,
    unroll: int = 1,
    wbufs: int = 4,
    cats_per_dma: int = 1,
    interleave: bool = False,
    alt_rings: bool = False,
    split_first: bool = False,
    w_engine: str = "sync",
    merge_xt: bool = False,
    ppbufs: int = 4,
    opbufs: int = 3,
    out_dtype: str = "float32",
    xbufs: int = 1,
    split_rings: bool = False,
    cengine: str = "vector",
    obatch: int = 1,
    contig: bool = False,
    dbg: str = "none",
    oeng: str = "scalar",
    xeng: str = "scalar",
    psplit: bool = False,
    xsplit: bool = False,
    wsplit: bool = False,
):
    """Build + compile the SPMD Bass program (same NEFF runs on all 8 cores).

    PC: per-slot sample capacities (even, <= 128) — an int (uniform) or a
        sequence of CPC values. Slot g on every core holds one category
        padded to PC[g] samples.
    loop_iters: if > 0, wrap the body in a device-side For_i loop with
        `unroll` copies of the body per iteration (timing use only).
    """
    import concourse.mybir as mybir
    import concourse.tile as tile
    from concourse import bacc

    f32 = mybir.dt.float32
    mmdt = getattr(mybir.dt, mm_dtype)
    odt = getattr(mybir.dt, out_dtype)
    PCs = [PC] * CPC if isinstance(PC, int) else list(PC)
    assert len(PCs) == CPC
    SOFF = [0]
    for p in PCs:
        SOFF.append(SOFF[-1] + p)
    NCOL = SOFF[-1]

    if contig:
        # Host pre-permutes weights/x/out so every DMA is a fully
        # contiguous 128-partition region (8 KB sequential runs per
        # partition — the max-bandwidth DMA geometry). Layouts:
        #   wt : [NBLK*128, G*KCH*OUT]  row b*128+p = [g][k][o] runs
        #   xt : [128, KCH*NCOL]        row p      = [k][col] runs
        #   out: [PCu, CPC*OUT]         row p      = [g][o] runs
        # where row p of chunk k maps IN index k*128+p (same for x & w).
        assert len(set(PCs)) == 1
        PCu = PCs[0]
        G = cats_per_dma
        NBLK = CPC // G
        nc = bacc.Bacc(
            "TRN2", target_bir_lowering=False, debug=False, num_devices=NCORES
        )
        wt = nc.dram_tensor(
            "wt", [NBLK * 128, G * KCH * OUT], mmdt, kind="ExternalInput"
        ).ap()
        xt = nc.dram_tensor(
            "xt", [128, KCH * NCOL], mmdt, kind="ExternalInput"
        ).ap()
        if psplit:
            # Even cats land on partitions 0..PCu-1, odd cats on 64..63+PCu
            # (PSUM base_partition 64). The two out DMAs then hit disjoint
            # SDMA-engine halves and run concurrently on separate rings.
            assert PCu <= 64 and obatch == CPC // 2
            out = nc.dram_tensor(
                "out", [128, (CPC // 2) * OUT], odt, kind="ExternalOutput"
            ).ap()
        else:
            out = nc.dram_tensor(
                "out", [PCu, CPC * OUT], odt, kind="ExternalOutput"
            ).ap()
        with tile.TileContext(nc) as tc:
            with (
                tc.tile_pool(name="xp", bufs=xbufs) as xp,
                tc.tile_pool(name="wp", bufs=wbufs) as wp,
                tc.tile_pool(name="pp", bufs=ppbufs, space="PSUM") as pp,
                tc.tile_pool(name="op", bufs=opbufs) as op,
            ):

                fixed = {}
                if dbg.startswith("mm_only"):
                    # Timing diagnostics: weights/x loaded ONCE outside the
                    # loop; body is compute-only (+ out DMA unless noout).
                    xf = xp.tile([128, KCH, NCOL], mmdt, tag="xf")
                    nc.scalar.dma_start(
                        out=xf[:], in_=xt.rearrange("p (k c) -> p k c", k=KCH)
                    )
                    fixed["x"] = xf
                    fixed["w"] = []
                    for b in range(NBLK):
                        wf = wp.tile([128, G, KCH, OUT], mmdt, tag=f"wf{b}")
                        nc.sync.dma_start(
                            out=wf[:],
                            in_=wt[b * 128 : (b + 1) * 128, :].rearrange(
                                "p (g k o) -> p g k o", g=G, k=KCH
                            ),
                        )
                        fixed["w"].append(wf)

                def body():
                    if dbg.startswith("mm_only"):
                        xt1 = fixed["x"]
                    else:
                        xt1 = xp.tile([128, KCH, NCOL], mmdt, tag="x1")
                        xsrc = xt.rearrange("p (k c) -> p k c", k=KCH)
                        if xsplit:
                            nc.sync.dma_start(
                                out=xt1[:, : KCH // 2, :],
                                in_=xsrc[:, : KCH // 2, :],
                            )
                            nc.scalar.dma_start(
                                out=xt1[:, KCH // 2 :, :],
                                in_=xsrc[:, KCH // 2 :, :],
                            )
                        else:
                            getattr(nc, xeng).dma_start(
                                out=xt1[:], in_=xsrc
                            )
                    for b in range(NBLK):
                        if dbg.startswith("mm_only"):
                            wtile = fixed["w"][b]
                        else:
                            wtile = wp.tile([128, G, KCH, OUT], mmdt)
                            wsrc = wt[b * 128 : (b + 1) * 128, :].rearrange(
                                "p (g k o) -> p g k o", g=G, k=KCH
                            )
                            if wsplit:
                                # halves of each block on both HWDGE rings
                                nc.sync.dma_start(
                                    out=wtile[:, :, : KCH // 2, :],
                                    in_=wsrc[:, :, : KCH // 2, :],
                                )
                                nc.scalar.dma_start(
                                    out=wtile[:, :, KCH // 2 :, :],
                                    in_=wsrc[:, :, KCH // 2 :, :],
                                )
                            else:
                                weng = (
                                    nc.scalar
                                    if (alt_rings and b % 2)
                                    else getattr(nc, w_engine)
                                )
                                weng.dma_start(out=wtile[:], in_=wsrc)
                        if dbg == "dma_only":
                            continue
                        for gl in range(G):
                            g = b * G + gl
                            base = 64 * (g % 2) if psplit else 0
                            if psplit:
                                psf = pp.tile([128, OUT], f32, tag="ps")
                                ps = psf[base : base + PCu, :]
                            else:
                                ps = pp.tile([PCu, OUT], f32, tag="ps")[:]
                            for k in range(KCH):
                                nc.tensor.matmul(
                                    ps,
                                    lhsT=xt1[:, k, SOFF[g] : SOFF[g] + PCu],
                                    rhs=wtile[:, gl, k, :],
                                    start=(k == 0),
                                    stop=(k == KCH - 1),
                                )
                            if g % obatch == 0 and not psplit:
                                body.obuf = op.tile(
                                    [PCu, obatch, OUT], odt, tag="ob"
                                )
                            elif psplit and g == 0:
                                body.obuf = op.tile(
                                    [128, obatch, OUT], odt, tag="ob"
                                )
                            cname = (
                                ("vector" if g % 2 else "scalar")
                                if cengine == "alt"
                                else cengine
                            )
                            if psplit:
                                dst = body.obuf[base : base + PCu, g // 2, :]
                            else:
                                dst = body.obuf[:, g % obatch, :]
                            if cname == "scalar":
                                nc.scalar.copy(out=dst, in_=ps)
                            else:
                                nc.vector.tensor_copy(out=dst, in_=ps)
                            if dbg == "mm_only_noout":
                                continue
                            if psplit:
                                if g == CPC - 2:  # last even cat done
                                    nc.sync.dma_start(
                                        out=out[0:PCu, :].rearrange(
                                            "p (g o) -> p g o", g=obatch
                                        ),
                                        in_=body.obuf[0:PCu, :, :],
                                    )
                                elif g == CPC - 1:  # last odd cat done
                                    nc.scalar.dma_start(
                                        out=out[64 : 64 + PCu, :].rearrange(
                                            "p (g o) -> p g o", g=obatch
                                        ),
                                        in_=body.obuf[64 : 64 + PCu, :, :],
                                    )
                            elif g % obatch == obatch - 1:
                                g0 = g - (obatch - 1)
                                getattr(nc, oeng).dma_start(
                                    out=out[
                                        :, g0 * OUT : (g0 + obatch) * OUT
                                    ].rearrange("p (g o) -> p g o", g=obatch),
                                    in_=body.obuf[:],
                                )

                if loop_iters > 0:
                    with tc.For_i(0, loop_iters, 1):
                        for _ in range(unroll):
                            body()
                else:
                    for _ in range(unroll):
                        body()
        nc.compile()
        return nc

    nc = bacc.Bacc(
        "TRN2", target_bir_lowering=False, debug=False, num_devices=NCORES
    )
    wt = nc.dram_tensor("wt", [CPC * IN, OUT], mmdt, kind="ExternalInput").ap()
    xt = nc.dram_tensor("xt", [IN, NCOL], mmdt, kind="ExternalInput").ap()
    out = nc.dram_tensor("out", [NCOL, OUT], odt, kind="ExternalOutput").ap()

    with tile.TileContext(nc) as tc:
        with (
            tc.tile_pool(name="xp", bufs=xbufs) as xp,
            tc.tile_pool(name="wp", bufs=wbufs) as wp,
            tc.tile_pool(name="pp", bufs=ppbufs, space="PSUM") as pp,
            tc.tile_pool(name="op", bufs=opbufs) as op,
        ):

            def body():
                G = cats_per_dma
                if interleave:
                    # p-outer row mapping: partition p holds IN rows
                    # KCH*p + s (s=0..KCH-1). Every DMA is contiguous per
                    # partition (8 KB weight runs, one single xT DMA); the
                    # contraction over s-subsets is a row permutation the
                    # matmul accumulation doesn't care about, as long as x
                    # and W use the same mapping.
                    xt4 = xp.tile([128, KCH, NCOL], mmdt, tag="x4")
                    nc.scalar.dma_start(
                        out=xt4[:], in_=xt.rearrange("(p s) c -> p s c", p=128)
                    )
                    lhs = lambda s, g: xt4[:, s, SOFF[g] : SOFF[g] + PCs[g]]
                elif merge_xt:
                    # One 3-D DMA for all four k-chunks (same k-outer layout,
                    # one descriptor chain / one fixed cost on the fill path).
                    xt1 = xp.tile([128, KCH, NCOL], mmdt, tag="x1")
                    nc.scalar.dma_start(
                        out=xt1[:], in_=xt.rearrange("(k p) c -> p k c", p=128)
                    )
                    lhs = lambda s, g: xt1[:, s, SOFF[g] : SOFF[g] + PCs[g]]
                else:
                    xts = []
                    for k in range(KCH):
                        t = xp.tile([128, NCOL], mmdt, tag=f"x{k}")
                        # ACT ring: keep SP HWDGE free for the weight stream
                        nc.scalar.dma_start(
                            out=t[:], in_=xt[k * 128 : (k + 1) * 128, :]
                        )
                        xts.append(t)
                    lhs = lambda s, g: xts[s][:, SOFF[g] : SOFF[g] + PCs[g]]
                for gp in range(0, CPC, G):
                    # Weight block [G cats] as SBUF [128, G, KCH, OUT]. G MB/DMA.
                    wtile = wp.tile([128, G, KCH, OUT], mmdt)
                    if interleave:
                        src = wt[gp * IN : (gp + G) * IN, :].rearrange(
                            "(g p s) o -> p g s o", p=128, s=KCH
                        )
                    else:
                        src = wt[gp * IN : (gp + G) * IN, :].rearrange(
                            "(g k p) o -> p g k o", p=128, k=KCH
                        )
                    weng = (
                        nc.scalar
                        if (alt_rings and (gp // G) % 2)
                        else getattr(nc, w_engine)
                    )
                    if split_rings:
                        # Both HWDGE rings stream halves of the same block
                        # concurrently (k-chunks 0-1 on SP, 2-3 on ACT).
                        nc.sync.dma_start(
                            out=wtile[:, :, : KCH // 2, :],
                            in_=src[:, :, : KCH // 2, :],
                        )
                        nc.scalar.dma_start(
                            out=wtile[:, :, KCH // 2 :, :],
                            in_=src[:, :, KCH // 2 :, :],
                        )
                        first_half = None
                    elif split_first and gp == 0 and G == 1:
                        # Halve the fill latency: the first two matmuls only
                        # need k-chunks 0-1, so land them in their own DMA.
                        half = wp.tile([128, 1, KCH // 2, OUT], mmdt, tag="wh")
                        weng.dma_start(
                            out=half[:],
                            in_=wt[0 : IN // 2, :].rearrange(
                                "(g k p) o -> p g k o", p=128, k=KCH // 2
                            ),
                        )
                        weng.dma_start(
                            out=wtile[:, :, KCH // 2 :, :],
                            in_=wt[IN // 2 : IN, :].rearrange(
                                "(g k p) o -> p g k o", p=128, k=KCH // 2
                            ),
                        )
                        first_half = half
                    else:
                        weng.dma_start(out=wtile[:], in_=src)
                        first_half = None
                    for gl in range(G):
                        g = gp + gl
                        ps = pp.tile([PCs[g], OUT], f32, tag="ps")
                        for k in range(KCH):
                            if first_half is not None and k < KCH // 2:
                                rhs = first_half[:, gl, k, :]
                            else:
                                rhs = wtile[:, gl, k, :]
                            nc.tensor.matmul(
                                ps[:],
                                lhsT=lhs(k, g),
                                rhs=rhs,
                                start=(k == 0),
                                stop=(k == KCH - 1),
                            )
                        cname = (
                            ("vector" if g % 2 else "scalar")
                            if cengine == "alt"
                            else cengine
                        )

                        def _pscopy(dst, src):
                            # ACT's copy op is `copy`; DVE's is `tensor_copy`
                            if cname == "scalar":
                                nc.scalar.copy(out=dst, in_=src)
                            else:
                                nc.vector.tensor_copy(out=dst, in_=src)

                        if obatch > 1:
                            # Coalesce obatch categories into one out DMA
                            # (uniform PC required for the row rearrange).
                            assert len(set(PCs)) == 1
                            if g % obatch == 0:
                                obuf = op.tile(
                                    [PCs[g], obatch, OUT], odt, tag="ob"
                                )
                                body.obuf = obuf
                            _pscopy(body.obuf[:, g % obatch, :], ps[:])
                            if g % obatch == obatch - 1:
                                g0 = g - (obatch - 1)
                                dst = out[
                                    SOFF[g0] : SOFF[g0] + obatch * PCs[g], :
                                ].rearrange("(g p) o -> p g o", p=PCs[g])
                                nc.scalar.dma_start(out=dst, in_=body.obuf[:])
                        else:
                            ot = op.tile([PCs[g], OUT], odt, tag="ot")
                            _pscopy(ot[:], ps[:])
                            nc.scalar.dma_start(
                                out=out[SOFF[g] : SOFF[g] + PCs[g], :], in_=ot[:]
                            )

            if loop_iters > 0:
                with tc.For_i(0, loop_iters, 1):
                    for _ in range(unroll):
                        body()
            else:
                for _ in range(unroll):
                    body()
    nc.compile()
    return nc


def _prepare(x, selected_ids, weight, mm_dtype="float32r", pc=None,
             contig=False, cats_per_dma=1):
    """Host-side shard prep. Returns (in_maps, meta), or (None, None) when the
    inputs don't fit the compiled layout (handled by the host fallback)."""
    host_dt = np.float16 if mm_dtype == "float16" else np.float32
    x = np.ascontiguousarray(np.asarray(x, dtype=np.float32))
    ids = np.asarray(selected_ids).astype(np.int64).ravel()
    weight = np.asarray(weight, dtype=np.float32)
    if ids.size != B or ids.min() < 0 or ids.max() >= C:
        return None, None  # out-of-range ids -> host path
    counts = np.bincount(ids, minlength=C)
    mx = int(counts.max())
    if mx > 128 or weight.shape != (C, OUT, IN) or x.shape != (B, IN):
        return None, None  # pathological skew / unexpected shape -> host path
    order = np.argsort(ids, kind="stable")
    x_sorted = x[order]
    offs = np.zeros(C + 1, np.int64)
    offs[1:] = np.cumsum(counts)
    # Identity assignment, uniform capacity rounded to 16. Measured fastest on
    # HW: sorted-assignment layouts with tighter per-slot capacities moved
    # ~0.5 MB/core less but ran 0.9-1.5 us slower (shorter DMA runs / smaller
    # output blocks cost more than the saved bytes). Capacity must be EVEN or
    # the fp32r matmul fast path degrades ~2x (PC=43 measured 58.8 us).
    assign = np.arange(C).reshape(NCORES, CPC).T  # [slot, core] -> category
    if pc is not None and mx <= pc <= 128 and pc % 2 == 0:
        PCs = [pc] * CPC
    else:
        PCs = [min(128, max(16, (mx + 15) // 16 * 16))] * CPC
    SOFF = np.zeros(CPC + 1, np.int64)
    SOFF[1:] = np.cumsum(PCs)
    NCOL = int(SOFF[-1])
    wt_t = np.ascontiguousarray(weight.transpose(0, 2, 1).astype(host_dt))
    in_maps = []
    G = cats_per_dma
    for core in range(NCORES):
        xt_k = np.zeros((IN, NCOL), host_dt)
        wlist = []
        for g in range(CPC):
            c = int(assign[g, core])
            n = int(counts[c])
            if n:
                xt_k[:, SOFF[g] : SOFF[g] + n] = (
                    x_sorted[offs[c] : offs[c + 1]].T.astype(host_dt)
                )
            if not contig:
                wlist.append(wt_t[c])
        if contig:
            # [p][k] row permutation + per-block concatenation so each
            # weight/x DMA reads one fully contiguous region.
            KC = IN // 128
            xt_k = np.ascontiguousarray(
                xt_k.reshape(KC, 128, NCOL).transpose(1, 0, 2)
            ).reshape(128, KC * NCOL)
            NBLK = CPC // G
            w_k = np.empty((NBLK * 128, G * KC * OUT), host_dt)
            for b in range(NBLK):
                blk = np.stack(
                    [
                        wt_t[int(assign[b * G + gl, core])]
                        .reshape(KC, 128, OUT)
                        .transpose(1, 0, 2)
                        for gl in range(G)
                    ],
                    axis=1,
                )  # [128, G, KC, OUT]
                w_k[b * 128 : (b + 1) * 128] = blk.reshape(128, G * KC * OUT)
        else:
            w_k = np.concatenate(wlist, axis=0)  # [CPC*IN, OUT]
        in_maps.append({"wt": w_k, "xt": xt_k})
    meta = dict(
        PCs=PCs, SOFF=SOFF, assign=assign, counts=counts, offs=offs,
        order=order, contig=contig,
    )
    return in_maps, meta


def _gather(results, meta):
    counts, offs, order = meta["counts"], meta["offs"], meta["order"]
    assign, SOFF = meta["assign"], meta["SOFF"]
    out_sorted = np.empty((B, OUT), np.float32)
    for core in range(NCORES):
        o = results[core]["out"]
        psplit = meta.get("contig") and o.shape[0] == 128
        if psplit:
            o = o.reshape(128, CPC // 2, OUT)
        elif meta.get("contig"):
            o = o.reshape(-1, CPC, OUT)  # [PCu, CPC, OUT]
        for g in range(CPC):
            c = int(assign[g, core])
            n = int(counts[c])
            if not n:
                continue
            if psplit:
                base = 64 * (g % 2)
                out_sorted[offs[c] : offs[c + 1]] = o[base : base + n, g // 2]
            elif meta.get("contig"):
                out_sorted[offs[c] : offs[c + 1]] = o[:n, g, :]
            else:
                out_sorted[offs[c] : offs[c + 1]] = o[SOFF[g] : SOFF[g] + n]
    out_full = np.empty_like(out_sorted)
    out_full[order] = out_sorted
    return out_full


_LAST = {}  # debug/test introspection: last built nc + shard maps

# Best measured config (loop-slope 16.2 us/body vs 32.2 us baseline):
#   fp16 matmul + fp16 out (fro 3.2e-4 vs f64; gate is 2e-2),
#   contiguous host-packed DMA layouts (one 0.5 MB fully-contiguous DMA
#   per category, alternating between the two HWDGE rings),
#   psplit: even cats' PSUM/out rows on partitions 0-47, odd on 64-111,
#   so the two out DMAs use disjoint SDMA-engine halves concurrently,
#   x DMA on the SWDGE (gpsimd) ring, double-buffered across bodies.
BEST_MM = "float16"
BEST_PREP = dict(contig=True, cats_per_dma=1)
BEST_BUILD = dict(
    contig=True, cats_per_dma=1, wbufs=8, alt_rings=True, obatch=4,
    opbufs=6, out_dtype="float16", psplit=True, xeng="gpsimd", xbufs=2,
)


def kernel(x, selected_ids, weight):
    in_maps, meta = _prepare(x, selected_ids, weight, mm_dtype=BEST_MM,
                             **BEST_PREP)
    if in_maps is None:
        # Host fallback for inputs outside the compiled layout's assumptions.
        ids = np.asarray(selected_ids).astype(np.int64).ravel()
        w = np.asarray(weight, dtype=np.float32)
        xx = np.asarray(x, dtype=np.float32).reshape(ids.size, -1)
        outf = np.empty((ids.size, w.shape[1]), np.float32)
        for c in np.unique(ids):
            m = ids == c
            outf[m] = xx[m] @ w[c].T
        return outf
    from concourse.bass_utils import run_bass_kernel_spmd

    build = dict(BEST_BUILD)
    if meta["PCs"][0] > 64:
        # psplit needs PC <= 64 (partition-half packing); rare skew fallback
        build.update(psplit=False, obatch=CPC)
    nc = _build_nc(meta["PCs"], mm_dtype=BEST_MM, **build)
    _LAST.update(nc=nc, in_maps=in_maps, meta=meta)
    res = run_bass_kernel_spmd(nc, in_maps, core_ids=list(range(NCORES)))
    return _gather(res.results, meta)



# revision 36
# speedup vs baseline: 2.5705x; 1.1373x over previous
"""Trainium2 Bass kernel for nn_CategoricalLinear (MoE-routing batched matvec).

Problem: out[b] = weight[selected_ids[b]] @ x[b]
  x: [2048, 512] f32, selected_ids: [2048] int, weight: [64, 512, 512] f32
  out: [2048, 512] f32

Strategy (category-sharded, NOT the data-parallel hint):
  - Host: stable-sort samples by category; category c's samples become a
    contiguous block. Transpose x so features lie on SBUF partitions.
  - Each of the 8 cores owns 8 categories (8 MB weight slab — the minimal
    1/8 slice of the 64 MB table) and ALL samples routed to them (~256).
  - Per category g: out_g[s, o] = sum_i x[s, i] * W_g[o, i] computed as
    4 accumulating PE matmuls: stationary = xT chunk [128(K=IN), PC(samples)],
    moving = W_g^T chunk [128(K=IN), 512(OUT)], PSUM [PC, 512].
    float32r data path -> full-rate PE (fp32 would stream at 1/4 rate).
  - Weight slab streamed per-category (1 MB DMAs) and double-buffered so the
    PE and the output path hide entirely under the weight DMA (~8 MB/core,
    the bandwidth floor for this sharding).
  - Host: unpad + inverse-permute rows back to the original sample order.

This is better than data-parallel replication: sharding the batch would make
every core read ~the whole 64 MB table (8x the aggregate HBM traffic) and
leaves ~4 samples per (core, category) matmul.
"""

import numpy as np

B, IN, OUT, C = 2048, 512, 512, 64
NCORES = 8
CPC = C // NCORES  # categories per core
KCH = IN // 128  # contraction chunks of 128


def _build_nc(
    PC,
    mm_dtype: str = "float32r",
    loop_iters: int = 0,
    unroll: int = 1,
    wbufs: int = 4,
    cats_per_dma: int = 1,
    interleave: bool = False,
    alt_rings: bool = False,
    split_first: bool = False,
    w_engine: str = "sync",
    merge_xt: bool = False,
    ppbufs: int = 4,
    opbufs: int = 3,
    out_dtype: str = "float32",
    xbufs: int = 1,
    split_rings: bool = False,
    cengine: str = "vector",
    obatch: int = 1,
    contig: bool = False,
    dbg: str = "none",
    oeng: str = "scalar",
    xeng: str = "scalar",
    psplit: bool = False,
    xsplit: bool = False,
    wsplit: bool = False,
):
    """Build + compile the SPMD Bass program (same NEFF runs on all 8 cores).

    PC: per-slot sample capacities (even, <= 128) — an int (uniform) or a
        sequence of CPC values. Slot g on every core holds one category
        padded to PC[g] samples.
    loop_iters: if > 0, wrap the body in a device-side For_i loop with
        `unroll` copies of the body per iteration (timing use only).
    """
    import concourse.mybir as mybir
    import concourse.tile as tile
    from concourse import bacc

    f32 = mybir.dt.float32
    mmdt = getattr(mybir.dt, mm_dtype)
    odt = getattr(mybir.dt, out_dtype)
    PCs = [PC] * CPC if isinstance(PC, int) else list(PC)
    assert len(PCs) == CPC
    SOFF = [0]
    for p in PCs:
        SOFF.append(SOFF[-1] + p)
    NCOL = SOFF[-1]

    if contig:
        # Host pre-permutes weights/x/out so every DMA is a fully
        # contiguous 128-partition region (8 KB sequential runs per
        # partition — the max-bandwidth DMA geometry). Layouts:
        #   wt : [NBLK*128, G*KCH*OUT]  row b*128+p = [g][k][o] runs
        #   xt : [128, KCH*NCOL]        row p      = [k][col] runs
        #   out: [PCu, CPC*OUT]         row p      = [g][o] runs
        # where row p of chunk k maps IN index k*128+p (same for x & w).
        assert len(set(PCs)) == 1
        PCu = PCs[0]
        G = cats_per_dma
        NBLK = CPC // G
        nc = bacc.Bacc(
            "TRN2", target_bir_lowering=False, debug=False, num_devices=NCORES
        )
        wt = nc.dram_tensor(
            "wt", [NBLK * 128, G * KCH * OUT], mmdt, kind="ExternalInput"
        ).ap()
        xt = nc.dram_tensor(
            "xt", [128, KCH * NCOL], mmdt, kind="ExternalInput"
        ).ap()
        if psplit:
            # Even cats land on partitions 0..PCu-1, odd cats on 64..63+PCu
            # (PSUM base_partition 64). The two out DMAs then hit disjoint
            # SDMA-engine halves and run concurrently on separate rings.
            assert PCu <= 64 and obatch == CPC // 2
            out = nc.dram_tensor(
                "out", [128, (CPC // 2) * OUT], odt, kind="ExternalOutput"
            ).ap()
        else:
            out = nc.dram_tensor(
                "out", [PCu, CPC * OUT], odt, kind="ExternalOutput"
            ).ap()
        with tile.TileContext(nc) as tc:
            with (
                tc.tile_pool(name="xp", bufs=xbufs) as xp,
                tc.tile_pool(name="wp", bufs=wbufs) as wp,
                tc.tile_pool(name="pp", bufs=ppbufs, space="PSUM") as pp,
                tc.tile_pool(name="op", bufs=opbufs) as op,
            ):

                fixed = {}
                if dbg.startswith("mm_only"):
                    # Timing diagnostics: weights/x loaded ONCE outside the
                    # loop; body is compute-only (+ out DMA unless noout).
                    xf = xp.tile([128, KCH, NCOL], mmdt, tag="xf")
                    nc.scalar.dma_start(
                        out=xf[:], in_=xt.rearrange("p (k c) -> p k c", k=KCH)
                    )
                    fixed["x"] = xf
                    fixed["w"] = []
                    for b in range(NBLK):
                        wf = wp.tile([128, G, KCH, OUT], mmdt, tag=f"wf{b}")
                        nc.sync.dma_start(
                            out=wf[:],
                            in_=wt[b * 128 : (b + 1) * 128, :].rearrange(
                                "p (g k o) -> p g k o", g=G, k=KCH
                            ),
                        )
                        fixed["w"].append(wf)

                def body():
                    if dbg.startswith("mm_only"):
                        xt1 = fixed["x"]
                    else:
                        xt1 = xp.tile([128, KCH, NCOL], mmdt, tag="x1")
                        xsrc = xt.rearrange("p (k c) -> p k c", k=KCH)
                        if xsplit:
                            nc.sync.dma_start(
                                out=xt1[:, : KCH // 2, :],
                                in_=xsrc[:, : KCH // 2, :],
                            )
                            nc.scalar.dma_start(
                                out=xt1[:, KCH // 2 :, :],
                                in_=xsrc[:, KCH // 2 :, :],
                            )
                        else:
                            getattr(nc, xeng).dma_start(
                                out=xt1[:], in_=xsrc
                            )
                    for b in range(NBLK):
                        if dbg.startswith("mm_only"):
                            wtile = fixed["w"][b]
                        else:
                            wtile = wp.tile([128, G, KCH, OUT], mmdt)
                            wsrc = wt[b * 128 : (b + 1) * 128, :].rearrange(
                                "p (g k o) -> p g k o", g=G, k=KCH
                            )
                            if wsplit:
                                # halves of each block on both HWDGE rings
                                nc.sync.dma_start(
                                    out=wtile[:, :, : KCH // 2, :],
                                    in_=wsrc[:, :, : KCH // 2, :],
                                )
                                nc.scalar.dma_start(
                                    out=wtile[:, :, KCH // 2 :, :],
                                    in_=wsrc[:, :, KCH // 2 :, :],
                                )
                            else:
                                weng = (
                                    nc.scalar
                                    if (alt_rings and b % 2)
                                    else getattr(nc, w_engine)
                                )
                                weng.dma_start(out=wtile[:], in_=wsrc)
                        if dbg == "dma_only":
                            continue
                        for gl in range(G):
                            g = b * G + gl
                            base = 64 * (g % 2) if psplit else 0
                            if psplit:
                                psf = pp.tile([128, OUT], f32, tag="ps")
                                ps = psf[base : base + PCu, :]
                            else:
                                ps = pp.tile([PCu, OUT], f32, tag="ps")[:]
                            for k in range(KCH):
                                nc.tensor.matmul(
                                    ps,
                                    lhsT=xt1[:, k, SOFF[g] : SOFF[g] + PCu],
                                    rhs=wtile[:, gl, k, :],
                                    start=(k == 0),
                                    stop=(k == KCH - 1),
                                )
                            if g % obatch == 0 and not psplit:
                                body.obuf = op.tile(
                                    [PCu, obatch, OUT], odt, tag="ob"
                                )
                            elif psplit and g == 0:
                                body.obuf = op.tile(
                                    [128, obatch, OUT], odt, tag="ob"
                                )
                            cname = (
                                ("vector" if g % 2 else "scalar")
                                if cengine == "alt"
                                else cengine
                            )
                            if psplit:
                                dst = body.obuf[base : base + PCu, g // 2, :]
                            else:
                                dst = body.obuf[:, g % obatch, :]
                            if cname == "scalar":
                                nc.scalar.copy(out=dst, in_=ps)
                            else:
                                nc.vector.tensor_copy(out=dst, in_=ps)
                            if dbg == "mm_only_noout":
                                continue
                            if psplit:
                                oe1, oe2 = (
                                    (nc.gpsimd, nc.gpsimd)
                                    if oeng == "gpsimd"
                                    else (nc.sync, nc.scalar)
                                )
                                if g == CPC - 2:  # last even cat done
                                    oe1.dma_start(
                                        out=out[0:PCu, :].rearrange(
                                            "p (g o) -> p g o", g=obatch
                                        ),
                                        in_=body.obuf[0:PCu, :, :],
                                    )
                                elif g == CPC - 1:  # last odd cat done
                                    oe2.dma_start(
                                        out=out[64 : 64 + PCu, :].rearrange(
                                            "p (g o) -> p g o", g=obatch
                                        ),
                                        in_=body.obuf[64 : 64 + PCu, :, :],
                                    )
                            elif g % obatch == obatch - 1:
                                g0 = g - (obatch - 1)
                                getattr(nc, oeng).dma_start(
                                    out=out[
                                        :, g0 * OUT : (g0 + obatch) * OUT
                                    ].rearrange("p (g o) -> p g o", g=obatch),
                                    in_=body.obuf[:],
                                )

                if loop_iters > 0:
                    with tc.For_i(0, loop_iters, 1):
                        for _ in range(unroll):
                            body()
                else:
                    for _ in range(unroll):
                        body()
        nc.compile()
        return nc

    nc = bacc.Bacc(
        "TRN2", target_bir_lowering=False, debug=False, num_devices=NCORES
    )
    wt = nc.dram_tensor("wt", [CPC * IN, OUT], mmdt, kind="ExternalInput").ap()
    xt = nc.dram_tensor("xt", [IN, NCOL], mmdt, kind="ExternalInput").ap()
    out = nc.dram_tensor("out", [NCOL, OUT], odt, kind="ExternalOutput").ap()

    with tile.TileContext(nc) as tc:
        with (
            tc.tile_pool(name="xp", bufs=xbufs) as xp,
            tc.tile_pool(name="wp", bufs=wbufs) as wp,
            tc.tile_pool(name="pp", bufs=ppbufs, space="PSUM") as pp,
            tc.tile_pool(name="op", bufs=opbufs) as op,
        ):

            def body():
                G = cats_per_dma
                if interleave:
                    # p-outer row mapping: partition p holds IN rows
                    # KCH*p + s (s=0..KCH-1). Every DMA is contiguous per
                    # partition (8 KB weight runs, one single xT DMA); the
                    # contraction over s-subsets is a row permutation the
                    # matmul accumulation doesn't care about, as long as x
                    # and W use the same mapping.
                    xt4 = xp.tile([128, KCH, NCOL], mmdt, tag="x4")
                    nc.scalar.dma_start(
                        out=xt4[:], in_=xt.rearrange("(p s) c -> p s c", p=128)
                    )
                    lhs = lambda s, g: xt4[:, s, SOFF[g] : SOFF[g] + PCs[g]]
                elif merge_xt:
                    # One 3-D DMA for all four k-chunks (same k-outer layout,
                    # one descriptor chain / one fixed cost on the fill path).
                    xt1 = xp.tile([128, KCH, NCOL], mmdt, tag="x1")
                    nc.scalar.dma_start(
                        out=xt1[:], in_=xt.rearrange("(k p) c -> p k c", p=128)
                    )
                    lhs = lambda s, g: xt1[:, s, SOFF[g] : SOFF[g] + PCs[g]]
                else:
                    xts = []
                    for k in range(KCH):
                        t = xp.tile([128, NCOL], mmdt, tag=f"x{k}")
                        # ACT ring: keep SP HWDGE free for the weight stream
                        nc.scalar.dma_start(
                            out=t[:], in_=xt[k * 128 : (k + 1) * 128, :]
                        )
                        xts.append(t)
                    lhs = lambda s, g: xts[s][:, SOFF[g] : SOFF[g] + PCs[g]]
                for gp in range(0, CPC, G):
                    # Weight block [G cats] as SBUF [128, G, KCH, OUT]. G MB/DMA.
                    wtile = wp.tile([128, G, KCH, OUT], mmdt)
                    if interleave:
                        src = wt[gp * IN : (gp + G) * IN, :].rearrange(
                            "(g p s) o -> p g s o", p=128, s=KCH
                        )
                    else:
                        src = wt[gp * IN : (gp + G) * IN, :].rearrange(
                            "(g k p) o -> p g k o", p=128, k=KCH
                        )
                    weng = (
                        nc.scalar
                        if (alt_rings and (gp // G) % 2)
                        else getattr(nc, w_engine)
                    )
                    if split_rings:
                        # Both HWDGE rings stream halves of the same block
                        # concurrently (k-chunks 0-1 on SP, 2-3 on ACT).
                        nc.sync.dma_start(
                            out=wtile[:, :, : KCH // 2, :],
                            in_=src[:, :, : KCH // 2, :],
                        )
                        nc.scalar.dma_start(
                            out=wtile[:, :, KCH // 2 :, :],
                            in_=src[:, :, KCH // 2 :, :],
                        )
                        first_half = None
                    elif split_first and gp == 0 and G == 1:
                        # Halve the fill latency: the first two matmuls only
                        # need k-chunks 0-1, so land them in their own DMA.
                        half = wp.tile([128, 1, KCH // 2, OUT], mmdt, tag="wh")
                        weng.dma_start(
                            out=half[:],
                            in_=wt[0 : IN // 2, :].rearrange(
                                "(g k p) o -> p g k o", p=128, k=KCH // 2
                            ),
                        )
                        weng.dma_start(
                            out=wtile[:, :, KCH // 2 :, :],
                            in_=wt[IN // 2 : IN, :].rearrange(
                                "(g k p) o -> p g k o", p=128, k=KCH // 2
                            ),
                        )
                        first_half = half
                    else:
                        weng.dma_start(out=wtile[:], in_=src)
                        first_half = None
                    for gl in range(G):
                        g = gp + gl
                        ps = pp.tile([PCs[g], OUT], f32, tag="ps")
                        for k in range(KCH):
                            if first_half is not None and k < KCH // 2:
                                rhs = first_half[:, gl, k, :]
                            else:
                                rhs = wtile[:, gl, k, :]
                            nc.tensor.matmul(
                                ps[:],
                                lhsT=lhs(k, g),
                                rhs=rhs,
                                start=(k == 0),
                                stop=(k == KCH - 1),
                            )
                        cname = (
                            ("vector" if g % 2 else "scalar")
                            if cengine == "alt"
                            else cengine
                        )

                        def _pscopy(dst, src):
                            # ACT's copy op is `copy`; DVE's is `tensor_copy`
                            if cname == "scalar":
                                nc.scalar.copy(out=dst, in_=src)
                            else:
                                nc.vector.tensor_copy(out=dst, in_=src)

                        if obatch > 1:
                            # Coalesce obatch categories into one out DMA
                            # (uniform PC required for the row rearrange).
                            assert len(set(PCs)) == 1
                            if g % obatch == 0:
                                obuf = op.tile(
                                    [PCs[g], obatch, OUT], odt, tag="ob"
                                )
                                body.obuf = obuf
                            _pscopy(body.obuf[:, g % obatch, :], ps[:])
                            if g % obatch == obatch - 1:
                                g0 = g - (obatch - 1)
                                dst = out[
                                    SOFF[g0] : SOFF[g0] + obatch * PCs[g], :
                                ].rearrange("(g p) o -> p g o", p=PCs[g])
                                nc.scalar.dma_start(out=dst, in_=body.obuf[:])
                        else:
                            ot = op.tile([PCs[g], OUT], odt, tag="ot")
                            _pscopy(ot[:], ps[:])
                            nc.scalar.dma_start(
                                out=out[SOFF[g] : SOFF[g] + PCs[g], :], in_=ot[:]
                            )

            if loop_iters > 0:
                with tc.For_i(0, loop_iters, 1):
                    for _ in range(unroll):
                        body()
            else:
                for _ in range(unroll):
                    body()
    nc.compile()
    return nc


def _prepare(x, selected_ids, weight, mm_dtype="float32r", pc=None,
             contig=False, cats_per_dma=1):
    """Host-side shard prep. Returns (in_maps, meta), or (None, None) when the
    inputs don't fit the compiled layout (handled by the host fallback)."""
    host_dt = np.float16 if mm_dtype == "float16" else np.float32
    x = np.ascontiguousarray(np.asarray(x, dtype=np.float32))
    ids = np.asarray(selected_ids).astype(np.int64).ravel()
    weight = np.asarray(weight, dtype=np.float32)
    if ids.size != B or ids.min() < 0 or ids.max() >= C:
        return None, None  # out-of-range ids -> host path
    counts = np.bincount(ids, minlength=C)
    mx = int(counts.max())
    if mx > 128 or weight.shape != (C, OUT, IN) or x.shape != (B, IN):
        return None, None  # pathological skew / unexpected shape -> host path
    order = np.argsort(ids, kind="stable")
    x_sorted = x[order]
    offs = np.zeros(C + 1, np.int64)
    offs[1:] = np.cumsum(counts)
    # Identity assignment, uniform capacity rounded to 16. Measured fastest on
    # HW: sorted-assignment layouts with tighter per-slot capacities moved
    # ~0.5 MB/core less but ran 0.9-1.5 us slower (shorter DMA runs / smaller
    # output blocks cost more than the saved bytes). Capacity must be EVEN or
    # the fp32r matmul fast path degrades ~2x (PC=43 measured 58.8 us).
    assign = np.arange(C).reshape(NCORES, CPC).T  # [slot, core] -> category
    if pc is not None and mx <= pc <= 128 and pc % 2 == 0:
        PCs = [pc] * CPC
    else:
        PCs = [min(128, max(16, (mx + 15) // 16 * 16))] * CPC
    SOFF = np.zeros(CPC + 1, np.int64)
    SOFF[1:] = np.cumsum(PCs)
    NCOL = int(SOFF[-1])
    wt_t = np.ascontiguousarray(weight.transpose(0, 2, 1).astype(host_dt))
    in_maps = []
    G = cats_per_dma
    for core in range(NCORES):
        xt_k = np.zeros((IN, NCOL), host_dt)
        wlist = []
        for g in range(CPC):
            c = int(assign[g, core])
            n = int(counts[c])
            if n:
                xt_k[:, SOFF[g] : SOFF[g] + n] = (
                    x_sorted[offs[c] : offs[c + 1]].T.astype(host_dt)
                )
            if not contig:
                wlist.append(wt_t[c])
        if contig:
            # [p][k] row permutation + per-block concatenation so each
            # weight/x DMA reads one fully contiguous region.
            KC = IN // 128
            xt_k = np.ascontiguousarray(
                xt_k.reshape(KC, 128, NCOL).transpose(1, 0, 2)
            ).reshape(128, KC * NCOL)
            NBLK = CPC // G
            w_k = np.empty((NBLK * 128, G * KC * OUT), host_dt)
            for b in range(NBLK):
                blk = np.stack(
                    [
                        wt_t[int(assign[b * G + gl, core])]
                        .reshape(KC, 128, OUT)
                        .transpose(1, 0, 2)
                        for gl in range(G)
                    ],
                    axis=1,
                )  # [128, G, KC, OUT]
                w_k[b * 128 : (b + 1) * 128] = blk.reshape(128, G * KC * OUT)
        else:
            w_k = np.concatenate(wlist, axis=0)  # [CPC*IN, OUT]
        in_maps.append({"wt": w_k, "xt": xt_k})
    meta = dict(
        PCs=PCs, SOFF=SOFF, assign=assign, counts=counts, offs=offs,
        order=order, contig=contig,
    )
    return in_maps, meta


def _gather(results, meta):
    counts, offs, order = meta["counts"], meta["offs"], meta["order"]
    assign, SOFF = meta["assign"], meta["SOFF"]
    out_sorted = np.empty((B, OUT), np.float32)
    for core in range(NCORES):
        o = results[core]["out"]
        psplit = meta.get("psplit_used", meta.get("contig") and o.shape[0] == 128)
        if psplit:
            o = o.reshape(128, CPC // 2, OUT)
        elif meta.get("contig"):
            o = o.reshape(-1, CPC, OUT)  # [PCu, CPC, OUT]
        for g in range(CPC):
            c = int(assign[g, core])
            n = int(counts[c])
            if not n:
                continue
            if psplit:
                base = 64 * (g % 2)
                out_sorted[offs[c] : offs[c + 1]] = o[base : base + n, g // 2]
            elif meta.get("contig"):
                out_sorted[offs[c] : offs[c + 1]] = o[:n, g, :]
            else:
                out_sorted[offs[c] : offs[c + 1]] = o[SOFF[g] : SOFF[g] + n]
    out_full = np.empty_like(out_sorted)
    out_full[order] = out_sorted
    return out_full


_LAST = {}  # debug/test introspection: last built nc + shard maps

# Best measured config (loop-slope 16.2 us/body vs 32.2 us baseline):
#   fp16 matmul + fp16 out (fro 3.2e-4 vs f64; gate is 2e-2),
#   contiguous host-packed DMA layouts (one 0.5 MB fully-contiguous DMA
#   per category, alternating between the two HWDGE rings),
#   psplit: even cats' PSUM/out rows on partitions 0-47, odd on 64-111,
#   so the two out DMAs use disjoint SDMA-engine halves concurrently,
#   x DMA on the SWDGE (gpsimd) ring, double-buffered across bodies.
BEST_MM = "float16"
BEST_PREP = dict(contig=True, cats_per_dma=1)
BEST_BUILD = dict(
    contig=True, cats_per_dma=1, wbufs=8, alt_rings=True, obatch=4,
    opbufs=6, out_dtype="float16", psplit=True, xeng="gpsimd", xbufs=2,
)


def kernel(x, selected_ids, weight):
    in_maps, meta = _prepare(x, selected_ids, weight, mm_dtype=BEST_MM,
                             **BEST_PREP)
    if in_maps is None:
        # Host fallback for inputs outside the compiled layout's assumptions.
        ids = np.asarray(selected_ids).astype(np.int64).ravel()
        w = np.asarray(weight, dtype=np.float32)
        xx = np.asarray(x, dtype=np.float32).reshape(ids.size, -1)
        outf = np.empty((ids.size, w.shape[1]), np.float32)
        for c in np.unique(ids):
            m = ids == c
            outf[m] = xx[m] @ w[c].T
        return outf
    from concourse.bass_utils import run_bass_kernel_spmd

    build = dict(BEST_BUILD)
    if meta["PCs"][0] > 64:
        # psplit needs PC <= 64 (partition-half packing); rare skew fallback
        build.update(psplit=False, obatch=CPC)
    meta["psplit_used"] = build["psplit"]
    nc = _build_nc(meta["PCs"], mm_dtype=BEST_MM, **build)
    _LAST.update(nc=nc, in_maps=in_maps, meta=meta)
    res = run_bass_kernel_spmd(nc, in_maps, core_ids=list(range(NCORES)))
    return _gather(res.results, meta)

